# revision 9
# baseline (speedup 1.0000x reference)
"""Multi-head attention (B=4, N=2048, C=768, H=12) on 8 trn2 NeuronCores.

Sharding: core c handles batch b = c//2 and query rows [ (c%2)*1024, +1024 ).
Each core computes K/V for its full batch (duplicated across the pair),
attention for all 12 heads over its 1024 queries, and the output projection
for its rows. Output gather is pure concatenation (no cross-core reduce).

Host side: all activations/weights are pre-cast to bf16 and x is column-
rotated per core so the core's own query columns are always x.T[:, 0:1024]
(key order is permuted identically in K and V, which leaves attention
invariant). This keeps one SPMD program for all 8 cores.

On-chip layout (per core):
  xt  = x_b.T (rolled)  [768, 2048]  bf16, c on partitions
  QT  = Wq @ xq.T       [768, 1024]  head h rows h*64..h*64+63
  KT  = Wk @ x.T        [768, 2048]
  V   = x @ Wv.T        [2048, 768]  row-tiles of 128 keys
  ST_h = (K_h Q_h^T)    [128k, 1024q] PSUM per k-tile; the two heads of a
                                     pair run concurrently in separate PE
                                     row groups (tile_position 0 / 64)
  exp on ScalarE with the 1/8 attention scale folded in -> pt (bf16)
  OT   = V^T P          [128, 1024]  ONE PSUM tile per pair: head0 on
                                     partitions 0..63 (col group 0), head1
                                     on 64..127 (col group 64) -> the two AV
                                     matmuls run concurrently in separate PE
                                     column groups
  den  = sum_k pt       ktile-sum on DVE (bf16 adds in 4x mode), then
                                     partition-sum+broadcast on GpSimd
                                     (partition_all_reduce, upcasts to f32)
  norm: reciprocal_approx_fast + multiply, full-width on the DVE
  Y    = OT^T Wp^T + bp [1024, 768]  bias added by the DVE during drain

Program order interleaves the per-pair linears (QT/KT/V chunks) between
attention pairs so the PE always has independent work while a pair's
denominators settle, and the Scalar engine (exp) starts as early as
possible. PSUM budget (8 banks): psa "a" 2x[128,1024] (ST tiles + proj)
+ psb "b" 2x[128,1024] (pair accumulators).
"""

import os
import sys

import numpy as np

sys.path.insert(0, "/opt/trn_rl_repo")

import concourse.bass as bass
from concourse import bacc
from concourse import bass_isa
import concourse.mybir as mybir
from concourse.tile import TileContext
from concourse.bass_utils import run_bass_kernel_spmd

P = 128
C = 768
NK = 2048
NQ = 1024
H = 12
DH = 64
CT = C // P          # 6 c-tiles (contraction tiles for the linears)
KT = NK // P         # 16 key tiles
QCH = 512            # max psum bank free dim for fp32
SCALE = DH ** -0.5
F32 = mybir.dt.float32
BF16 = mybir.dt.bfloat16

LAST_RESULT = None
_PROG = None


def _build_program() -> bass.Bass:
    nc = bacc.Bacc(None, target_bir_lowering=False)

    xt = nc.dram_tensor("xt", [C, NK], BF16, kind="ExternalInput")
    wqt = nc.dram_tensor("wqt", [C, C], BF16, kind="ExternalInput")
    wkt = nc.dram_tensor("wkt", [C, C], BF16, kind="ExternalInput")
    wvt = nc.dram_tensor("wvt", [C, C], BF16, kind="ExternalInput")
    wpt = nc.dram_tensor("wpt", [C, C], BF16, kind="ExternalInput")
    bpb = nc.dram_tensor("bpb", [P, C], F32, kind="ExternalInput")
    y = nc.dram_tensor("y", [NQ, C], F32, kind="ExternalOutput")

    with TileContext(nc) as tc:
        with (
            tc.tile_pool(name="persist", bufs=1) as persist,
            tc.tile_pool(name="pt", bufs=6) as ptp,
            tc.tile_pool(name="small", bufs=2) as small,
            tc.tile_pool(name="ysb", bufs=2) as ysb,
            tc.tile_pool(name="psa", bufs=2, space="PSUM") as psa,
            tc.tile_pool(name="psb", bufs=2, space="PSUM") as psb,
        ):
            # ---- persistent SBUF tiles ----
            def load(dram, rows, cols, tag):
                tiles = []
                for i in range(rows // P):
                    t = persist.tile([P, cols], BF16, tag=f"{tag}{i}",
                                     name=f"{tag}{i}")
                    nc.sync.dma_start(out=t[:, :], in_=dram[i * P:(i + 1) * P, :])
                    tiles.append(t)
                return tiles

            # load order: wq + xt unblock QT, then wk, wv, wp, bias
            wqb = load(wqt, C, C, "wqb")
            xtb = load(xt, C, NK, "xtb")
            wkb = load(wkt, C, C, "wkb")
            wvb = load(wvt, C, C, "wvb")
            wpb = load(wpt, C, C, "wpb")
            bpf = persist.tile([P, C], F32, tag="bpf", name="bpf")
            nc.sync.dma_start(out=bpf[:, :], in_=bpb[:, :])

            qtb = [persist.tile([P, NQ], BF16, tag=f"qt{i}", name=f"qt{i}")
                   for i in range(CT)]
            ktb = [persist.tile([P, NK], BF16, tag=f"kt{i}", name=f"kt{i}")
                   for i in range(CT)]
            vtb = [persist.tile([P, C], BF16, tag=f"v{i}", name=f"v{i}")
                   for i in range(KT)]
            otb = [persist.tile([P, NQ], BF16, tag=f"ot{i}", name=f"ot{i}")
                   for i in range(CT)]

            # ---- linears, emitted per cout-tile so they interleave ----
            def qt_tile(i):
                ps = psa.tile([P, NQ], F32, tag="a", name="qps")
                for j in range(2):
                    for k in range(CT):
                        nc.tensor.matmul(
                            ps[:, j * QCH:(j + 1) * QCH],
                            lhsT=wqb[k][:, i * P:(i + 1) * P],
                            rhs=xtb[k][:, j * QCH:(j + 1) * QCH],
                            start=(k == 0),
                            stop=(k == CT - 1),
                        )
                nc.vector.tensor_copy(qtb[i][:, :], ps[:, :])

            def kt_tile(i):
                for half in range(2):
                    ps = psa.tile([P, NQ], F32, tag="a", name="kps")
                    for j in range(2):
                        c0 = half * NQ + j * QCH
                        for k in range(CT):
                            nc.tensor.matmul(
                                ps[:, j * QCH:(j + 1) * QCH],
                                lhsT=wkb[k][:, i * P:(i + 1) * P],
                                rhs=xtb[k][:, c0:c0 + QCH],
                                start=(k == 0),
                                stop=(k == CT - 1),
                            )
                    nc.vector.tensor_copy(
                        ktb[i][:, half * NQ:(half + 1) * NQ], ps[:, :])

            def v_chunk(c):
                # V columns c*256..c*256+255 = heads 4c..4c+3 for all k-tiles
                for i in range(KT):
                    ps = psa.tile([P, 4 * DH], F32, tag="a", name="vps")
                    for k in range(CT):
                        nc.tensor.matmul(
                            ps[:, :],
                            lhsT=xtb[k][:, i * P:(i + 1) * P],
                            rhs=wvb[k][:, c * 4 * DH:(c + 1) * 4 * DH],
                            start=(k == 0),
                            stop=(k == CT - 1),
                        )
                    nc.vector.tensor_copy(
                        vtb[i][:, c * 4 * DH:(c + 1) * 4 * DH], ps[:, :])

            # ---- attention pair ----
            def attention_pair(hp):
                h0, h1 = 2 * hp, 2 * hp + 1
                ot0 = psb.tile([DH, NQ], F32, tag="b", name="ot0")
                ot1 = psb.tile([DH, NQ], F32, tag="b", name="ot1")
                acc0 = small.tile([P, NQ], BF16, tag="acc0", name="acc0")
                acc1 = small.tile([P, NQ], BF16, tag="acc1", name="acc1")

                def av_pair(i, pt0, pt1):
                    for j in range(2):
                        nc.tensor.matmul(
                            ot0[:, j * QCH:(j + 1) * QCH],
                            lhsT=vtb[i][:, h0 * DH:(h0 + 1) * DH],
                            rhs=pt0[:, j * QCH:(j + 1) * QCH],
                            start=(i == 0), stop=(i == KT - 1),
                        )
                        nc.tensor.matmul(
                            ot1[:, j * QCH:(j + 1) * QCH],
                            lhsT=vtb[i][:, h1 * DH:(h1 + 1) * DH],
                            rhs=pt1[:, j * QCH:(j + 1) * QCH],
                            start=(i == 0), stop=(i == KT - 1),
                        )

                # software pipeline: AV(i-1) is emitted between ST(i) and
                # exp(i), so the PE never stalls on the exp it just fed
                pending = None
                first = None
                for i in range(KT):
                    st0 = psa.tile([P, NQ], F32, tag="a", name="st0")
                    st1 = psa.tile([P, NQ], F32, tag="a", name="st1")
                    for j in range(2):
                        # heads alternate PE row groups (base 0 / base 64)
                        # -> hardware runs the pair concurrently
                        nc.tensor.matmul(
                            st0[:, j * QCH:(j + 1) * QCH],
                            lhsT=ktb[hp][0:DH, i * P:(i + 1) * P],
                            rhs=qtb[hp][0:DH, j * QCH:(j + 1) * QCH],
                            start=True, stop=True,
                            tile_position=(0, 0),
                        )
                        nc.tensor.matmul(
                            st1[:, j * QCH:(j + 1) * QCH],
                            lhsT=ktb[hp][DH:P, i * P:(i + 1) * P],
                            rhs=qtb[hp][DH:P, j * QCH:(j + 1) * QCH],
                            start=True, stop=True,
                            tile_position=(64, 0),
                        )
                    if pending is not None:
                        av_pair(*pending)
                    pt0 = ptp.tile([P, NQ], BF16, tag="pt", name="pt0")
                    pt1 = ptp.tile([P, NQ], BF16, tag="pt", name="pt1")
                    nc.scalar.activation(
                        pt0[:, :], st0[:, :],
                        mybir.ActivationFunctionType.Exp, scale=SCALE,
                    )
                    nc.scalar.activation(
                        pt1[:, :], st1[:, :],
                        mybir.ActivationFunctionType.Exp, scale=SCALE,
                    )
                    # denominator k-tile sums: bf16 adds run in the DVE 4x
                    # mode, ~0.27us each
                    if i == 1:
                        nc.vector.tensor_add(acc0[:, :], first[0][:, :], pt0[:, :])
                        nc.vector.tensor_add(acc1[:, :], first[1][:, :], pt1[:, :])
                    elif i >= 2:
                        nc.vector.tensor_add(acc0[:, :], acc0[:, :], pt0[:, :])
                        nc.vector.tensor_add(acc1[:, :], acc1[:, :], pt1[:, :])
                    else:
                        first = (pt0, pt1)
                    pending = (i, pt0, pt1)
                av_pair(*pending)
                return (hp, ot0, ot1, acc0, acc1)

            def norm(hp, ot0, ot1, acc0, acc1):
                # partition sum + broadcast of the denominators (f32 accum)
                db0 = small.tile([P, NQ], F32, tag="db", name="db0")
                nc.gpsimd.partition_all_reduce(
                    db0[:, :], acc0[:, :], channels=P,
                    reduce_op=bass_isa.ReduceOp.add)
                db1 = small.tile([P, NQ], F32, tag="db", name="db1")
                nc.gpsimd.partition_all_reduce(
                    db1[:, :], acc1[:, :], channels=P,
                    reduce_op=bass_isa.ReduceOp.add)
                # reciprocal_approx_fast needs full-width (128-partition)
                # operands; db0/db1 are partition-broadcast so that's free
                rc0 = small.tile([P, NQ], F32, tag="rc0", name="rc0")
                nc.vector.reciprocal_approx_fast(rc0[:, :], db0[:, :])
                rc1 = small.tile([P, NQ], F32, tag="rc1", name="rc1")
                nc.vector.reciprocal_approx_fast(rc1[:, :], db1[:, :])
                # drain numerators (partition-shifting copies are safe; a
                # partition-shifting multi-input multiply is not)
                osb = small.tile([P, NQ], F32, tag="osb", name="osb")
                nc.vector.tensor_copy(osb[0:DH, :], ot0[:, :])
                nc.vector.tensor_copy(osb[DH:P, :], ot1[:, :])
                nc.vector.tensor_mul(otb[hp][0:DH, :], osb[0:DH, :], rc0[0:DH, :])
                nc.vector.tensor_mul(otb[hp][DH:P, :], osb[DH:P, :], rc1[DH:P, :])

            # ---- interleaved schedule ----
            qt_tile(0)
            kt_tile(0)
            v_chunk(0)                      # heads 0..3 (pairs 0, 1)
            prev = attention_pair(0)
            qt_tile(1)
            kt_tile(1)
            norm(*prev)
            prev = attention_pair(1)
            v_chunk(1)                      # heads 4..7 (pairs 2, 3)
            qt_tile(2)
            kt_tile(2)
            norm(*prev)
            prev = attention_pair(2)
            qt_tile(3)
            kt_tile(3)
            norm(*prev)
            prev = attention_pair(3)
            v_chunk(2)                      # heads 8..11 (pairs 4, 5)
            qt_tile(4)
            kt_tile(4)
            norm(*prev)
            prev = attention_pair(4)
            qt_tile(5)
            kt_tile(5)
            norm(*prev)
            prev = attention_pair(5)
            norm(*prev)

            # ---- projection: Y[q, co] = OT.T @ WpT + bp ----
            for qi in range(NQ // P):
                yt = ysb.tile([P, C], F32, tag="y", name="yt")
                for (c0, csz) in ((0, QCH), (QCH, C - QCH)):
                    ps = psa.tile([P, csz], F32, tag="a", name="pps")
                    for k in range(CT):
                        nc.tensor.matmul(
                            ps[:, :],
                            lhsT=otb[k][:, qi * P:(qi + 1) * P],
                            rhs=wpb[k][:, c0:c0 + csz],
                            start=(k == 0), stop=(k == CT - 1),
                        )
                    nc.vector.tensor_add(
                        yt[:, c0:c0 + csz], ps[:, :], bpf[:, c0:c0 + csz])
                nc.sync.dma_start(out=y[qi * P:(qi + 1) * P, :], in_=yt[:, :])

    nc.compile()
    return nc


def _get_prog() -> bass.Bass:
    global _PROG
    if _PROG is None:
        _PROG = _build_program()
    return _PROG


def kernel(x, Wq, Wk, Wv, Wp, bp):
    global LAST_RESULT
    import ml_dtypes
    bf16 = ml_dtypes.bfloat16

    x = np.asarray(x, dtype=np.float32)
    wqt = np.ascontiguousarray(np.asarray(Wq, np.float32).T).astype(bf16)
    wkt = np.ascontiguousarray(np.asarray(Wk, np.float32).T).astype(bf16)
    wvt = np.ascontiguousarray(np.asarray(Wv, np.float32).T).astype(bf16)
    wpt = np.ascontiguousarray(np.asarray(Wp, np.float32).T).astype(bf16)
    bpv = np.ascontiguousarray(np.broadcast_to(
        np.asarray(bp, np.float32).reshape(1, C), (P, C)))

    B, N, _ = x.shape
    in_maps = []
    for core in range(8):
        b, qh = core // 2, core % 2
        # roll the key columns so this core's queries are columns 0..1023;
        # K and V see the same permutation, so attention is unchanged
        xtf = x[b].T
        xtr = np.concatenate(
            [xtf[:, qh * NQ:], xtf[:, :qh * NQ]], axis=1)
        in_maps.append({
            "xt": np.ascontiguousarray(xtr).astype(bf16),
            "wqt": wqt, "wkt": wkt, "wvt": wvt, "wpt": wpt, "bpb": bpv,
        })

    res = run_bass_kernel_spmd(
        _get_prog(), in_maps, core_ids=list(range(8)),
        trace=bool(os.environ.get("BASS_TRACE")),
    )
    LAST_RESULT = res

    out = np.empty((B, N, C), np.float32)
    for core in range(8):
        b, qh = core // 2, core % 2
        out[b, qh * NQ:(qh + 1) * NQ, :] = res.results[core]["y"]
    return out


# revision 11
# speedup vs baseline: 1.1480x; 1.1480x over previous
"""Multi-head attention (B=4, N=2048, C=768, H=12) on 8 trn2 NeuronCores.

Sharding: core c handles batch b = c//2 and query rows [ (c%2)*1024, +1024 ).
Each core computes K/V for its full batch (duplicated across the pair),
attention for all 12 heads over its 1024 queries, and the output projection
for its rows. Output gather is pure concatenation (no cross-core reduce).

Host side: all activations/weights are pre-cast to bf16 and x is column-
rotated per core so the core's own query columns are always x.T[:, 0:1024]
(key order is permuted identically in K and V, which leaves attention
invariant). This keeps one SPMD program for all 8 cores.

On-chip layout (per core):
  xt  = x_b.T (rolled)  [768, 2048]  bf16, c on partitions
  QT  = Wq @ xq.T       [768, 1024]  head h rows h*64..h*64+63
  KT  = Wk @ x.T        [768, 2048]
  V   = x @ Wv.T        [2048, 768]  row-tiles of 128 keys
  ST_h = (K_h Q_h^T)    [128k, 1024q] PSUM per k-tile; the two heads of a
                                     pair run concurrently in separate PE
                                     row groups (tile_position 0 / 64)
  exp on ScalarE with the 1/8 attention scale folded in -> pt (bf16)
  OT   = V^T P          [128, 1024]  ONE PSUM tile per pair: head0 on
                                     partitions 0..63 (col group 0), head1
                                     on 64..127 (col group 64) -> the two AV
                                     matmuls run concurrently in separate PE
                                     column groups
  den  = sum_k pt       ktile-sum on DVE (bf16 adds in 4x mode), then
                                     partition-sum+broadcast on GpSimd
                                     (partition_all_reduce, upcasts to f32)
  norm: reciprocal_approx_fast + multiply, full-width on the DVE
  Y    = OT^T Wp^T + bp [1024, 768]  bias added by the DVE during drain

Program order interleaves the per-pair linears (QT/KT/V chunks) between
attention pairs so the PE always has independent work while a pair's
denominators settle, and the Scalar engine (exp) starts as early as
possible. PSUM budget (8 banks): psa "a" 2x[128,1024] (ST tiles + proj)
+ psb "b" 2x[128,1024] (pair accumulators).
"""

import os
import sys

import numpy as np

sys.path.insert(0, "/opt/trn_rl_repo")

import concourse.bass as bass
from concourse import bacc
from concourse import bass_isa
import concourse.mybir as mybir
from concourse.tile import TileContext
from concourse.bass_utils import run_bass_kernel_spmd

P = 128
C = 768
NK = 2048
NQ = 1024
H = 12
DH = 64
CT = C // P          # 6 c-tiles (contraction tiles for the linears)
KT = NK // P         # 16 key tiles
QCH = 512            # max psum bank free dim for fp32
SCALE = DH ** -0.5
F32 = mybir.dt.float32
BF16 = mybir.dt.bfloat16

LAST_RESULT = None
_PROG = None


def _build_program() -> bass.Bass:
    nc = bacc.Bacc(None, target_bir_lowering=False)

    xt = nc.dram_tensor("xt", [C, NK], BF16, kind="ExternalInput")
    wqt = nc.dram_tensor("wqt", [C, C], BF16, kind="ExternalInput")
    wkt = nc.dram_tensor("wkt", [C, C], BF16, kind="ExternalInput")
    wvt = nc.dram_tensor("wvt", [C, C], BF16, kind="ExternalInput")
    wpt = nc.dram_tensor("wpt", [C, C], BF16, kind="ExternalInput")
    bpb = nc.dram_tensor("bpb", [P, C], F32, kind="ExternalInput")
    y = nc.dram_tensor("y", [NQ, C], F32, kind="ExternalOutput")

    with TileContext(nc) as tc:
        with (
            tc.tile_pool(name="persist", bufs=1) as persist,
            tc.tile_pool(name="pt", bufs=6) as ptp,
            tc.tile_pool(name="small", bufs=2) as small,
            tc.tile_pool(name="ysb", bufs=2) as ysb,
            tc.tile_pool(name="psa", bufs=2, space="PSUM") as psa,
            tc.tile_pool(name="psb", bufs=2, space="PSUM") as psb,
        ):
            # ---- persistent SBUF tiles ----
            def load(dram, rows, cols, tag):
                tiles = []
                for i in range(rows // P):
                    t = persist.tile([P, cols], BF16, tag=f"{tag}{i}",
                                     name=f"{tag}{i}")
                    nc.sync.dma_start(out=t[:, :], in_=dram[i * P:(i + 1) * P, :])
                    tiles.append(t)
                return tiles

            # load order: wq + xt unblock QT, then wk, wv, wp, bias
            wqb = load(wqt, C, C, "wqb")
            xtb = load(xt, C, NK, "xtb")
            wkb = load(wkt, C, C, "wkb")
            wvb = load(wvt, C, C, "wvb")
            wpb = load(wpt, C, C, "wpb")
            bpf = persist.tile([P, C], F32, tag="bpf", name="bpf")
            nc.sync.dma_start(out=bpf[:, :], in_=bpb[:, :])

            qtb = [persist.tile([P, NQ], BF16, tag=f"qt{i}", name=f"qt{i}")
                   for i in range(CT)]
            ktb = [persist.tile([P, NK], BF16, tag=f"kt{i}", name=f"kt{i}")
                   for i in range(CT)]
            vtb = [persist.tile([P, C], BF16, tag=f"v{i}", name=f"v{i}")
                   for i in range(KT)]
            otb = [persist.tile([P, NQ], BF16, tag=f"ot{i}", name=f"ot{i}")
                   for i in range(CT)]

            # ---- linears, emitted per cout-tile so they interleave ----
            def qt_tile(i):
                ps = psa.tile([P, NQ], F32, tag="a", name="qps")
                for j in range(2):
                    for k in range(CT):
                        nc.tensor.matmul(
                            ps[:, j * QCH:(j + 1) * QCH],
                            lhsT=wqb[k][:, i * P:(i + 1) * P],
                            rhs=xtb[k][:, j * QCH:(j + 1) * QCH],
                            start=(k == 0),
                            stop=(k == CT - 1),
                        )
                nc.vector.tensor_copy(qtb[i][:, :], ps[:, :])

            def kt_tile(i):
                for half in range(2):
                    ps = psa.tile([P, NQ], F32, tag="a", name="kps")
                    for j in range(2):
                        c0 = half * NQ + j * QCH
                        for k in range(CT):
                            nc.tensor.matmul(
                                ps[:, j * QCH:(j + 1) * QCH],
                                lhsT=wkb[k][:, i * P:(i + 1) * P],
                                rhs=xtb[k][:, c0:c0 + QCH],
                                start=(k == 0),
                                stop=(k == CT - 1),
                            )
                    nc.vector.tensor_copy(
                        ktb[i][:, half * NQ:(half + 1) * NQ], ps[:, :])

            def v_chunk(c):
                # V columns c*256..c*256+255 = heads 4c..4c+3 for all k-tiles
                for i in range(KT):
                    ps = psa.tile([P, 4 * DH], F32, tag="a", name="vps")
                    for k in range(CT):
                        nc.tensor.matmul(
                            ps[:, :],
                            lhsT=xtb[k][:, i * P:(i + 1) * P],
                            rhs=wvb[k][:, c * 4 * DH:(c + 1) * 4 * DH],
                            start=(k == 0),
                            stop=(k == CT - 1),
                        )
                    nc.vector.tensor_copy(
                        vtb[i][:, c * 4 * DH:(c + 1) * 4 * DH], ps[:, :])

            # ---- attention pair ----
            def attention_pair(hp):
                h0, h1 = 2 * hp, 2 * hp + 1
                # one accumulator for both heads: head0 -> partitions 0..63
                # (PE col group 0), head1 -> 64..127 (col group 64); the two
                # AV matmuls run concurrently in separate PE column groups
                ot = psb.tile([P, NQ], F32, tag="b", name="ot")
                acc0 = small.tile([P, NQ], BF16, tag="acc0", name="acc0")
                acc1 = small.tile([P, NQ], BF16, tag="acc1", name="acc1")

                def av_pair(i, pt0, pt1):
                    for j in range(2):
                        nc.tensor.matmul(
                            ot[0:DH, j * QCH:(j + 1) * QCH],
                            lhsT=vtb[i][:, h0 * DH:(h0 + 1) * DH],
                            rhs=pt0[:, j * QCH:(j + 1) * QCH],
                            start=(i == 0), stop=(i == KT - 1),
                            tile_position=(0, 0),
                        )
                        nc.tensor.matmul(
                            ot[DH:P, j * QCH:(j + 1) * QCH],
                            lhsT=vtb[i][:, h1 * DH:(h1 + 1) * DH],
                            rhs=pt1[:, j * QCH:(j + 1) * QCH],
                            start=(i == 0), stop=(i == KT - 1),
                            tile_position=(0, 64),
                        )

                # software pipeline: AV(i-1) is emitted between ST(i) and
                # exp(i), so the PE never stalls on the exp it just fed
                pending = None
                first = None
                for i in range(KT):
                    st0 = psa.tile([P, NQ], F32, tag="a", name="st0")
                    st1 = psa.tile([P, NQ], F32, tag="a", name="st1")
                    for j in range(2):
                        # heads alternate PE row groups (base 0 / base 64)
                        # -> hardware runs the pair concurrently
                        nc.tensor.matmul(
                            st0[:, j * QCH:(j + 1) * QCH],
                            lhsT=ktb[hp][0:DH, i * P:(i + 1) * P],
                            rhs=qtb[hp][0:DH, j * QCH:(j + 1) * QCH],
                            start=True, stop=True,
                            tile_position=(0, 0),
                        )
                        nc.tensor.matmul(
                            st1[:, j * QCH:(j + 1) * QCH],
                            lhsT=ktb[hp][DH:P, i * P:(i + 1) * P],
                            rhs=qtb[hp][DH:P, j * QCH:(j + 1) * QCH],
                            start=True, stop=True,
                            tile_position=(64, 0),
                        )
                    if pending is not None:
                        av_pair(*pending)
                    pt0 = ptp.tile([P, NQ], BF16, tag="pt", name="pt0")
                    pt1 = ptp.tile([P, NQ], BF16, tag="pt", name="pt1")
                    nc.scalar.activation(
                        pt0[:, :], st0[:, :],
                        mybir.ActivationFunctionType.Exp, scale=SCALE,
                    )
                    nc.scalar.activation(
                        pt1[:, :], st1[:, :],
                        mybir.ActivationFunctionType.Exp, scale=SCALE,
                    )
                    # denominator k-tile sums: bf16 adds run in the DVE 4x
                    # mode, ~0.27us each
                    if i == 1:
                        nc.vector.tensor_add(acc0[:, :], first[0][:, :], pt0[:, :])
                        nc.vector.tensor_add(acc1[:, :], first[1][:, :], pt1[:, :])
                    elif i >= 2:
                        nc.vector.tensor_add(acc0[:, :], acc0[:, :], pt0[:, :])
                        nc.vector.tensor_add(acc1[:, :], acc1[:, :], pt1[:, :])
                    else:
                        first = (pt0, pt1)
                    pending = (i, pt0, pt1)
                av_pair(*pending)
                return (hp, ot, acc0, acc1)

            def norm(hp, ot, acc0, acc1):
                # partition sum + broadcast of the denominators (f32 accum)
                db0 = small.tile([P, NQ], F32, tag="db", name="db0")
                nc.gpsimd.partition_all_reduce(
                    db0[:, :], acc0[:, :], channels=P,
                    reduce_op=bass_isa.ReduceOp.add)
                db1 = small.tile([P, NQ], F32, tag="db", name="db1")
                nc.gpsimd.partition_all_reduce(
                    db1[:, :], acc1[:, :], channels=P,
                    reduce_op=bass_isa.ReduceOp.add)
                # reciprocal_approx_fast needs full-width (128-partition)
                # operands; db0/db1 are partition-broadcast so that's free
                rc0 = small.tile([P, NQ], F32, tag="rc0", name="rc0")
                nc.vector.reciprocal_approx_fast(rc0[:, :], db0[:, :])
                rc1 = small.tile([P, NQ], F32, tag="rc1", name="rc1")
                nc.vector.reciprocal_approx_fast(rc1[:, :], db1[:, :])
                # normalize straight out of PSUM (all operands base-aligned)
                nc.vector.tensor_mul(otb[hp][0:DH, :], ot[0:DH, :], rc0[0:DH, :])
                nc.vector.tensor_mul(otb[hp][DH:P, :], ot[DH:P, :], rc1[DH:P, :])

            # ---- interleaved schedule ----
            qt_tile(0)
            kt_tile(0)
            v_chunk(0)                      # heads 0..3 (pairs 0, 1)
            prev = attention_pair(0)
            qt_tile(1)
            kt_tile(1)
            norm(*prev)
            prev = attention_pair(1)
            v_chunk(1)                      # heads 4..7 (pairs 2, 3)
            qt_tile(2)
            kt_tile(2)
            norm(*prev)
            prev = attention_pair(2)
            qt_tile(3)
            kt_tile(3)
            norm(*prev)
            prev = attention_pair(3)
            v_chunk(2)                      # heads 8..11 (pairs 4, 5)
            qt_tile(4)
            kt_tile(4)
            norm(*prev)
            prev = attention_pair(4)
            qt_tile(5)
            kt_tile(5)
            norm(*prev)
            prev = attention_pair(5)
            norm(*prev)

            # ---- projection: Y[q, co] = OT.T @ WpT + bp ----
            for qi in range(NQ // P):
                yt = ysb.tile([P, C], F32, tag="y", name="yt")
                for (c0, csz) in ((0, QCH), (QCH, C - QCH)):
                    ps = psa.tile([P, csz], F32, tag="a", name="pps")
                    for k in range(CT):
                        nc.tensor.matmul(
                            ps[:, :],
                            lhsT=otb[k][:, qi * P:(qi + 1) * P],
                            rhs=wpb[k][:, c0:c0 + csz],
                            start=(k == 0), stop=(k == CT - 1),
                        )
                    nc.vector.tensor_add(
                        yt[:, c0:c0 + csz], ps[:, :], bpf[:, c0:c0 + csz])
                nc.sync.dma_start(out=y[qi * P:(qi + 1) * P, :], in_=yt[:, :])

    nc.compile()
    return nc


def _get_prog() -> bass.Bass:
    global _PROG
    if _PROG is None:
        _PROG = _build_program()
    return _PROG


def kernel(x, Wq, Wk, Wv, Wp, bp):
    global LAST_RESULT
    import ml_dtypes
    bf16 = ml_dtypes.bfloat16

    x = np.asarray(x, dtype=np.float32)
    wqt = np.ascontiguousarray(np.asarray(Wq, np.float32).T).astype(bf16)
    wkt = np.ascontiguousarray(np.asarray(Wk, np.float32).T).astype(bf16)
    wvt = np.ascontiguousarray(np.asarray(Wv, np.float32).T).astype(bf16)
    wpt = np.ascontiguousarray(np.asarray(Wp, np.float32).T).astype(bf16)
    bpv = np.ascontiguousarray(np.broadcast_to(
        np.asarray(bp, np.float32).reshape(1, C), (P, C)))

    B, N, _ = x.shape
    in_maps = []
    for core in range(8):
        b, qh = core // 2, core % 2
        # roll the key columns so this core's queries are columns 0..1023;
        # K and V see the same permutation, so attention is unchanged
        xtf = x[b].T
        xtr = np.concatenate(
            [xtf[:, qh * NQ:], xtf[:, :qh * NQ]], axis=1)
        in_maps.append({
            "xt": np.ascontiguousarray(xtr).astype(bf16),
            "wqt": wqt, "wkt": wkt, "wvt": wvt, "wpt": wpt, "bpb": bpv,
        })

    res = run_bass_kernel_spmd(
        _get_prog(), in_maps, core_ids=list(range(8)),
        trace=bool(os.environ.get("BASS_TRACE")),
    )
    LAST_RESULT = res

    out = np.empty((B, N, C), np.float32)
    for core in range(8):
        b, qh = core // 2, core % 2
        out[b, qh * NQ:(qh + 1) * NQ, :] = res.results[core]["y"]
    return out
